# revision 7
# baseline (speedup 1.0000x reference)
"""Trainium2 Bass kernel for CompactPiecewiseLinearEmbeddings.

out[n, f*8+d] = sum_b h[n,f,b] * W[f,b,d] + b[f,d]
h = piecewise-linear encoding of x[n,f] over per-feature bins
    (first bin clamp_max(1), middle clamp(0,1), last clamp_min(0)).

Strategy (per core; data-parallel over N across 8 cores):
 - PE "broadcast matmul" with a fixed signed selector puts +-x onto 96
   partition rows per feature pair (2 features x 48 bins, with the two
   special rows for bin0/bin47 first).
 - ACT (scaled relu-affine) or DVE (subtract/max) builds the unclamped
   basis values; GPSIMD/DVE applies (scale+)min-1 clamp to middle rows.
   bin0's clamp_max-only semantics use relu(c1 - x) with negated weight.
 - PE block-diagonal matmul (lhsT = packed [96,16] weights per pair,
   bf16) contracts to out[16, n] slices stacked 8 pairs -> [128, n].
 - ACT evacuates + adds bias; PE transposes to [n, fd]; DMA out.
"""
import numpy as np
import ml_dtypes

from concourse import bacc, mybir
from concourse.tile import TileContext
from concourse.bass_utils import run_bass_kernel_spmd

N, F, B, D = 16384, 256, 48, 8
NCORES = 8
NS = N // NCORES          # 2048 rows per core
CH = 512                  # n-chunk
NCH = NS // CH            # 4
NPAIR = F // 2            # 128 feature pairs
NGRP = NPAIR // 8         # 16 groups of 8 pairs
KROWS = 96                # rows per pair: 2 special bin0 + 2 bin47 + 92 middle

_cache = {}


def build_nc():
    nc = bacc.Bacc("TRN2")
    f32, f16, bf16 = mybir.dt.float32, mybir.dt.float16, mybir.dt.bfloat16

    x_ext = nc.declare_dram_parameter("x", [NS, F], f32, isOutput=False)
    sel_ext = nc.declare_dram_parameter("sel", [128, 64 * KROWS], f16, isOutput=False)
    ident_ext = nc.declare_dram_parameter("ident", [128, 128], f32, isOutput=False)
    scaleA_ext = nc.declare_dram_parameter("scaleA", [KROWS, NPAIR], f32, isOutput=False)
    biasA_ext = nc.declare_dram_parameter("biasA", [KROWS, NPAIR], f32, isOutput=False)
    eD_ext = nc.declare_dram_parameter("eD", [KROWS, NPAIR], f32, isOutput=False)
    winvD_ext = nc.declare_dram_parameter("winvD", [KROWS, NPAIR + 1], f32, isOutput=False)
    bound_ext = nc.declare_dram_parameter("bound", [KROWS, 1], f32, isOutput=False)
    wpack_ext = nc.declare_dram_parameter("wpack", [KROWS, NPAIR * 64], bf16, isOutput=False)
    obias_ext = nc.declare_dram_parameter("obias", [128, NGRP], f32, isOutput=False)
    out_ext = nc.declare_dram_parameter("out", [NS, F * D], f32, isOutput=True)

    with TileContext(nc) as tc:
        with (
            tc.tile_pool(name="const", bufs=1) as cpool,
            tc.tile_pool(name="hbuf", bufs=4) as hpool,
            tc.tile_pool(name="osb", bufs=NGRP + 2) as opool,
            tc.tile_pool(name="fin", bufs=3) as fpool,
            tc.tile_pool(name="bc", bufs=3, space="PSUM") as bcpool,
            tc.tile_pool(name="oc", bufs=2, space="PSUM") as ocpool,
            tc.tile_pool(name="tp", bufs=2, space="PSUM") as tppool,
        ):
            # ---- load constants ----
            sel = cpool.tile([128, 64 * KROWS], f16)
            ident = cpool.tile([128, 128], f32)
            scaleA = cpool.tile([KROWS, NPAIR], f32)
            biasA = cpool.tile([KROWS, NPAIR], f32)
            eD = cpool.tile([KROWS, NPAIR], f32)
            winvD = cpool.tile([KROWS, NPAIR + 1], f32)
            bound = cpool.tile([KROWS, 1], f32)
            wpack = cpool.tile([KROWS, NPAIR * 64], bf16)
            obias = cpool.tile([128, NGRP], f32)
            for t, e in [(sel, sel_ext), (ident, ident_ext), (scaleA, scaleA_ext),
                         (biasA, biasA_ext), (eD, eD_ext), (winvD, winvD_ext), (bound, bound_ext),
                         (wpack, wpack_ext), (obias, obias_ext)]:
                nc.sync.dma_start(out=t[:], in_=e[:])

            # ---- load x and build x_T in fp16: two [128, NS] tiles ----
            xsb = cpool.tile([128, 16 * F], f32)          # 16 row-tiles side by side
            for nt in range(16):
                nc.sync.dma_start(out=xsb[:, nt * F:(nt + 1) * F],
                                  in_=x_ext[nt * 128:(nt + 1) * 128, :])
            xT = [cpool.tile([128, NS], f16, tag=f"xT{t}", name=f"xT{t}") for t in range(2)]
            for nt in range(16):
                for ft in range(2):
                    tp = tppool.tile([128, 512], f32, tag="tp")
                    nc.tensor.transpose(
                        tp[:, 0:128],
                        xsb[:, nt * F + ft * 128: nt * F + ft * 128 + 128],
                        ident[:],
                    )
                    nc.vector.tensor_copy(xT[ft][:, nt * 128:(nt + 1) * 128],
                                          tp[:, 0:128])

            # ---- main loop ----
            for c in range(NCH):
                osb_tiles = []
                for g in range(NGRP):
                    oc = ocpool.tile([128, CH], f32, tag="oc")
                    for i in range(8):
                        fp = g * 8 + i
                        ft, prel = (2 * fp) // 128, (2 * fp) % 128
                        bc = bcpool.tile([KROWS, CH], f32, tag="bc")
                        fpr = prel // 2
                        nc.tensor.matmul(
                            bc[:],
                            sel[:, fpr * KROWS:(fpr + 1) * KROWS],
                            xT[ft][:, c * CH:(c + 1) * CH],
                            start=True, stop=True,
                        )
                        h = hpool.tile([KROWS, CH], bf16, tag="h")
                        route_act = (i % 2 == 0)
                        if route_act:
                            nc.scalar.activation(
                                h[:], bc[:], mybir.ActivationFunctionType.Relu,
                                bias=biasA[:, fp:fp + 1], scale=scaleA[:, fp:fp + 1],
                            )
                        else:
                            nc.vector.tensor_scalar(
                                h[:], bc[:], eD[:, fp:fp + 1], 0.0,
                                mybir.AluOpType.subtract, mybir.AluOpType.max,
                            )
                        # clamp middle rows; scale first for DVE route.
                        # out-of-place full tile: specials pass through via
                        # scale 1.0 and bound +inf.
                        p2_on_gp = (i % 4) != 1
                        eng2 = nc.gpsimd if p2_on_gp else nc.vector
                        h2 = hpool.tile([KROWS, CH], bf16, tag="h2")
                        scol = NPAIR if route_act else fp
                        eng2.tensor_scalar(
                            h2[:], h[:],
                            winvD[:, scol:scol + 1], bound[:, 0:1],
                            mybir.AluOpType.mult, mybir.AluOpType.min,
                        )
                        nc.tensor.matmul(
                            oc[(i // 4) * 64:(i // 4) * 64 + 64, :],
                            wpack[:, fp * 64:(fp + 1) * 64],
                            h2[:],
                            start=(i % 4 == 0), stop=(i % 4 == 3),
                        )
                    osb = opool.tile([128, CH], f32, tag="osb")
                    nc.scalar.activation(
                        osb[:], oc[:], mybir.ActivationFunctionType.Identity,
                        bias=obias[:, g:g + 1],
                    )
                    osb_tiles.append(osb)
                # transpose 16 groups x [128, CH] -> 4 final [128, 2048] tiles
                for nt in range(CH // 128):
                    fin = fpool.tile([128, F * D], f32, tag="fin")
                    for q in range(4):
                        tp = tppool.tile([128, 512], f32, tag="tp")
                        for j in range(4):
                            g = q * 4 + j
                            nc.tensor.transpose(
                                tp[:, j * 128:(j + 1) * 128],
                                osb_tiles[g][:, nt * 128:(nt + 1) * 128],
                                ident[:],
                            )
                        nc.vector.tensor_copy(fin[:, q * 512:(q + 1) * 512], tp[:])
                    r0 = c * CH + nt * 128
                    nc.sync.dma_start(out=out_ext[r0:r0 + 128, :], in_=fin[:])

    nc.compile()
    return nc


def host_constants(edges, width, W, b):
    """Build packed constant tensors. edges/width [F,B], W [F,B,D], b [F,D]."""
    f32 = np.float32
    winv = (1.0 / width).astype(f32)
    c1 = (edges[:, 0] + width[:, 0]).astype(f32)      # second boundary
    # row order per pair: [g0'(f0), g0'(f1), j47(f0), j47(f1),
    #                      f0 j=1..46 (4..49), f1 j=1..46 (50..95)]
    sel2 = np.zeros((2, KROWS), np.float16)
    sel2[0, 0:46] = 1.0
    sel2[1, 46:92] = 1.0
    sel2[0, 92] = -1.0
    sel2[1, 93] = -1.0
    sel2[0, 94] = 1.0
    sel2[1, 95] = 1.0
    sel = np.zeros((128, 64 * KROWS), np.float16)
    for fpr in range(64):
        sel[2 * fpr:2 * fpr + 2, fpr * KROWS:(fpr + 1) * KROWS] = sel2

    scaleA = np.zeros((KROWS, NPAIR), f32)
    biasA = np.zeros((KROWS, NPAIR), f32)
    eD = np.zeros((KROWS, NPAIR), f32)
    winvD = np.ones((KROWS, NPAIR + 1), f32)
    bound = np.full((KROWS, 1), 1.0, f32)
    bound[92:96, 0] = 3.0e38
    wpack = np.zeros((KROWS, NPAIR * 64), f32)
    obias = np.zeros((128, NGRP), f32)

    for fp in range(NPAIR):
        for k in range(2):
            f = 2 * fp + k
            # bin0 special row: value relu(c1 - x); coeff -W0*winv0; bias +W0
            r = 92 + k
            scaleA[r, fp] = 1.0
            biasA[r, fp] = c1[f]
            eD[r, fp] = -c1[f]
            half = (fp % 4) * 16
            wpack[r, fp * 64 + half + k * 8:fp * 64 + half + k * 8 + 8] = -W[f, 0, :] * winv[f, 0]
            # bin47 row: value relu(x - e47); coeff W47*winv47
            r = 94 + k
            scaleA[r, fp] = 1.0
            biasA[r, fp] = -edges[f, 47]
            eD[r, fp] = edges[f, 47]
            wpack[r, fp * 64 + half + k * 8:fp * 64 + half + k * 8 + 8] = W[f, 47, :] * winv[f, 47]
            # middle rows j=1..46
            for j in range(1, 47):
                r = k * 46 + (j - 1)
                scaleA[r, fp] = winv[f, j]
                biasA[r, fp] = -edges[f, j] * winv[f, j]
                eD[r, fp] = edges[f, j]
                winvD[r, fp] = winv[f, j]
                wpack[r, fp * 64 + half + k * 8:fp * 64 + half + k * 8 + 8] = W[f, j, :]
        g, i = fp // 8, fp % 8
        for k in range(2):
            f = 2 * fp + k
            obias[16 * i + k * 8:16 * i + k * 8 + 8, g] = b[f, :] + W[f, 0, :]

    consts = {
        "sel": sel,
        "ident": np.eye(128, dtype=f32),
        "scaleA": scaleA,
        "biasA": biasA,
        "eD": eD,
        "winvD": winvD,
        "bound": bound,
        "wpack": wpack.astype(ml_dtypes.bfloat16),
        "obias": obias,
    }
    return consts


def make_in_maps(x, edges, width, W, b):
    consts = host_constants(np.asarray(edges), np.asarray(width),
                            np.asarray(W), np.asarray(b))
    x = np.ascontiguousarray(np.asarray(x, dtype=np.float32))
    in_maps = []
    for core in range(NCORES):
        m = dict(consts)
        m["x"] = x[core * NS:(core + 1) * NS, :]
        in_maps.append(m)
    return in_maps


def kernel(x, edges, width, W, b):
    if "nc" not in _cache:
        _cache["nc"] = build_nc()
    nc = _cache["nc"]
    in_maps = make_in_maps(x, edges, width, W, b)
    res = run_bass_kernel_spmd(nc, in_maps, core_ids=list(range(NCORES)))
    outs = [np.asarray(r["out"]) for r in res.results]
    return np.concatenate(outs, axis=0)


# revision 8
# speedup vs baseline: 1.1934x; 1.1934x over previous
"""Trainium2 Bass kernel for CompactPiecewiseLinearEmbeddings.

out[n, f*8+d] = sum_b h[n,f,b] * W[f,b,d] + b[f,d]
h = piecewise-linear encoding of x[n,f] over per-feature bins
    (first bin clamp_max(1), middle clamp(0,1), last clamp_min(0)).

Strategy (per core; data-parallel over N across 8 cores):
 - PE "broadcast matmul" with a fixed signed selector puts +-x onto 96
   partition rows per feature pair (2 features x 48 bins, with the two
   special rows for bin0/bin47 first).
 - ACT (scaled relu-affine) or DVE (subtract/max) builds the unclamped
   basis values; GPSIMD/DVE applies (scale+)min-1 clamp to middle rows.
   bin0's clamp_max-only semantics use relu(c1 - x) with negated weight.
 - PE block-diagonal matmul (lhsT = packed [96,16] weights per pair,
   bf16) contracts to out[16, n] slices stacked 8 pairs -> [128, n].
 - ACT evacuates + adds bias; PE transposes to [n, fd]; DMA out.
"""
import numpy as np
import ml_dtypes

from concourse import bacc, mybir
from concourse.tile import TileContext
from concourse.bass_utils import run_bass_kernel_spmd

N, F, B, D = 16384, 256, 48, 8
NCORES = 8
NS = N // NCORES          # 2048 rows per core
CH = 512                  # n-chunk
NCH = NS // CH            # 4
NPAIR = F // 2            # 128 feature pairs
NGRP = NPAIR // 8         # 16 groups of 8 pairs
KROWS = 96                # rows per pair: 2 special bin0 + 2 bin47 + 92 middle

_cache = {}


def build_nc():
    nc = bacc.Bacc("TRN2")
    f32, f16, bf16 = mybir.dt.float32, mybir.dt.float16, mybir.dt.bfloat16

    x_ext = nc.declare_dram_parameter("x", [NS, F], f32, isOutput=False)
    sel_ext = nc.declare_dram_parameter("sel", [128, 64 * KROWS], f16, isOutput=False)
    ident_ext = nc.declare_dram_parameter("ident", [128, 128], f32, isOutput=False)
    scaleA_ext = nc.declare_dram_parameter("scaleA", [KROWS, NPAIR], f32, isOutput=False)
    biasA_ext = nc.declare_dram_parameter("biasA", [KROWS, NPAIR], f32, isOutput=False)
    eD_ext = nc.declare_dram_parameter("eD", [KROWS, NPAIR], f32, isOutput=False)
    winvD_ext = nc.declare_dram_parameter("winvD", [KROWS, NPAIR + 1], f32, isOutput=False)
    bound_ext = nc.declare_dram_parameter("bound", [KROWS, 1], f32, isOutput=False)
    wpack_ext = nc.declare_dram_parameter("wpack", [KROWS, NPAIR * 64], bf16, isOutput=False)
    obias_ext = nc.declare_dram_parameter("obias", [128, NGRP], f32, isOutput=False)
    out_ext = nc.declare_dram_parameter("out", [NS, F * D], f32, isOutput=True)

    with TileContext(nc) as tc:
        with (
            tc.tile_pool(name="const", bufs=1) as cpool,
            tc.tile_pool(name="hbuf", bufs=8) as hpool,
            tc.tile_pool(name="osb", bufs=NGRP + 2) as opool,
            tc.tile_pool(name="fin", bufs=3) as fpool,
            tc.tile_pool(name="bc", bufs=4, space="PSUM") as bcpool,
            tc.tile_pool(name="oc", bufs=2, space="PSUM") as ocpool,
            tc.tile_pool(name="tp", bufs=2, space="PSUM") as tppool,
        ):
            # ---- load constants ----
            sel = cpool.tile([128, 64 * KROWS], f16)
            ident = cpool.tile([128, 128], f32)
            scaleA = cpool.tile([KROWS, NPAIR], f32)
            biasA = cpool.tile([KROWS, NPAIR], f32)
            eD = cpool.tile([KROWS, NPAIR], f32)
            winvD = cpool.tile([KROWS, NPAIR + 1], f32)
            bound = cpool.tile([KROWS, 1], f32)
            wpack = cpool.tile([KROWS, NPAIR * 64], bf16)
            obias = cpool.tile([128, NGRP], f32)
            for t, e in [(sel, sel_ext), (ident, ident_ext), (scaleA, scaleA_ext),
                         (biasA, biasA_ext), (eD, eD_ext), (winvD, winvD_ext), (bound, bound_ext),
                         (wpack, wpack_ext), (obias, obias_ext)]:
                nc.sync.dma_start(out=t[:], in_=e[:])

            # ---- load x and build x_T in fp16: two [128, NS] tiles ----
            xsb = cpool.tile([128, 16 * F], f32)          # 16 row-tiles side by side
            for nt in range(16):
                nc.sync.dma_start(out=xsb[:, nt * F:(nt + 1) * F],
                                  in_=x_ext[nt * 128:(nt + 1) * 128, :])
            xT = [cpool.tile([128, NS], f16, tag=f"xT{t}", name=f"xT{t}") for t in range(2)]
            for nt in range(16):
                for ft in range(2):
                    tp = tppool.tile([128, 512], f32, tag="tp")
                    nc.tensor.transpose(
                        tp[:, 0:128],
                        xsb[:, nt * F + ft * 128: nt * F + ft * 128 + 128],
                        ident[:],
                    )
                    nc.vector.tensor_copy(xT[ft][:, nt * 128:(nt + 1) * 128],
                                          tp[:, 0:128])

            # ---- main loop ----
            for c in range(NCH):
                osb_tiles = []
                for g in range(NGRP):
                    oc = ocpool.tile([128, CH], f32, tag="oc")
                    for i in range(8):
                        fp = g * 8 + i
                        ft, prel = (2 * fp) // 128, (2 * fp) % 128
                        bc = bcpool.tile([KROWS, CH], f32, tag="bc")
                        fpr = prel // 2
                        nc.tensor.matmul(
                            bc[:],
                            sel[:, fpr * KROWS:(fpr + 1) * KROWS],
                            xT[ft][:, c * CH:(c + 1) * CH],
                            start=True, stop=True,
                        )
                        h = hpool.tile([KROWS, CH], bf16, tag="h")
                        route_act = (i % 2 == 0)
                        if route_act:
                            nc.scalar.activation(
                                h[:], bc[:], mybir.ActivationFunctionType.Relu,
                                bias=biasA[:, fp:fp + 1], scale=scaleA[:, fp:fp + 1],
                            )
                        else:
                            nc.vector.tensor_scalar(
                                h[:], bc[:], eD[:, fp:fp + 1], 0.0,
                                mybir.AluOpType.subtract, mybir.AluOpType.max,
                            )
                        # clamp middle rows; scale first for DVE route.
                        # out-of-place full tile: specials pass through via
                        # scale 1.0 and bound +inf.
                        p2_on_gp = (i % 4) != 1
                        eng2 = nc.gpsimd if p2_on_gp else nc.vector
                        h2 = hpool.tile([KROWS, CH], bf16, tag="h2")
                        scol = NPAIR if route_act else fp
                        eng2.tensor_scalar(
                            h2[:], h[:],
                            winvD[:, scol:scol + 1], bound[:, 0:1],
                            mybir.AluOpType.mult, mybir.AluOpType.min,
                        )
                        nc.tensor.matmul(
                            oc[(i // 4) * 64:(i // 4) * 64 + 64, :],
                            wpack[:, fp * 64:(fp + 1) * 64],
                            h2[:],
                            start=(i % 4 == 0), stop=(i % 4 == 3),
                        )
                    osb = opool.tile([128, CH], f32, tag="osb")
                    nc.scalar.activation(
                        osb[:], oc[:], mybir.ActivationFunctionType.Identity,
                        bias=obias[:, g:g + 1],
                    )
                    osb_tiles.append(osb)
                # transpose 16 groups x [128, CH] -> 4 final [128, 2048] tiles
                for nt in range(CH // 128):
                    fin = fpool.tile([128, F * D], f32, tag="fin")
                    for q in range(4):
                        tp = tppool.tile([128, 512], f32, tag="tp")
                        for j in range(4):
                            g = q * 4 + j
                            nc.tensor.transpose(
                                tp[:, j * 128:(j + 1) * 128],
                                osb_tiles[g][:, nt * 128:(nt + 1) * 128],
                                ident[:],
                            )
                        nc.vector.tensor_copy(fin[:, q * 512:(q + 1) * 512], tp[:])
                    r0 = c * CH + nt * 128
                    nc.sync.dma_start(out=out_ext[r0:r0 + 128, :], in_=fin[:])

    nc.compile()
    return nc


def host_constants(edges, width, W, b):
    """Build packed constant tensors. edges/width [F,B], W [F,B,D], b [F,D]."""
    f32 = np.float32
    winv = (1.0 / width).astype(f32)
    c1 = (edges[:, 0] + width[:, 0]).astype(f32)      # second boundary
    # row order per pair: [g0'(f0), g0'(f1), j47(f0), j47(f1),
    #                      f0 j=1..46 (4..49), f1 j=1..46 (50..95)]
    sel2 = np.zeros((2, KROWS), np.float16)
    sel2[0, 0:46] = 1.0
    sel2[1, 46:92] = 1.0
    sel2[0, 92] = -1.0
    sel2[1, 93] = -1.0
    sel2[0, 94] = 1.0
    sel2[1, 95] = 1.0
    sel = np.zeros((128, 64 * KROWS), np.float16)
    for fpr in range(64):
        sel[2 * fpr:2 * fpr + 2, fpr * KROWS:(fpr + 1) * KROWS] = sel2

    scaleA = np.zeros((KROWS, NPAIR), f32)
    biasA = np.zeros((KROWS, NPAIR), f32)
    eD = np.zeros((KROWS, NPAIR), f32)
    winvD = np.ones((KROWS, NPAIR + 1), f32)
    bound = np.full((KROWS, 1), 1.0, f32)
    bound[92:96, 0] = 3.0e38
    wpack = np.zeros((KROWS, NPAIR * 64), f32)
    obias = np.zeros((128, NGRP), f32)

    for fp in range(NPAIR):
        for k in range(2):
            f = 2 * fp + k
            # bin0 special row: value relu(c1 - x); coeff -W0*winv0; bias +W0
            r = 92 + k
            scaleA[r, fp] = 1.0
            biasA[r, fp] = c1[f]
            eD[r, fp] = -c1[f]
            half = (fp % 4) * 16
            wpack[r, fp * 64 + half + k * 8:fp * 64 + half + k * 8 + 8] = -W[f, 0, :] * winv[f, 0]
            # bin47 row: value relu(x - e47); coeff W47*winv47
            r = 94 + k
            scaleA[r, fp] = 1.0
            biasA[r, fp] = -edges[f, 47]
            eD[r, fp] = edges[f, 47]
            wpack[r, fp * 64 + half + k * 8:fp * 64 + half + k * 8 + 8] = W[f, 47, :] * winv[f, 47]
            # middle rows j=1..46
            for j in range(1, 47):
                r = k * 46 + (j - 1)
                scaleA[r, fp] = winv[f, j]
                biasA[r, fp] = -edges[f, j] * winv[f, j]
                eD[r, fp] = edges[f, j]
                winvD[r, fp] = winv[f, j]
                wpack[r, fp * 64 + half + k * 8:fp * 64 + half + k * 8 + 8] = W[f, j, :]
        g, i = fp // 8, fp % 8
        for k in range(2):
            f = 2 * fp + k
            obias[16 * i + k * 8:16 * i + k * 8 + 8, g] = b[f, :] + W[f, 0, :]

    consts = {
        "sel": sel,
        "ident": np.eye(128, dtype=f32),
        "scaleA": scaleA,
        "biasA": biasA,
        "eD": eD,
        "winvD": winvD,
        "bound": bound,
        "wpack": wpack.astype(ml_dtypes.bfloat16),
        "obias": obias,
    }
    return consts


def make_in_maps(x, edges, width, W, b):
    consts = host_constants(np.asarray(edges), np.asarray(width),
                            np.asarray(W), np.asarray(b))
    x = np.ascontiguousarray(np.asarray(x, dtype=np.float32))
    in_maps = []
    for core in range(NCORES):
        m = dict(consts)
        m["x"] = x[core * NS:(core + 1) * NS, :]
        in_maps.append(m)
    return in_maps


def kernel(x, edges, width, W, b):
    if "nc" not in _cache:
        _cache["nc"] = build_nc()
    nc = _cache["nc"]
    in_maps = make_in_maps(x, edges, width, W, b)
    res = run_bass_kernel_spmd(nc, in_maps, core_ids=list(range(NCORES)))
    outs = [np.asarray(r["out"]) for r in res.results]
    return np.concatenate(outs, axis=0)
